# revision 16
# baseline (speedup 1.0000x reference)
"""Trainium2 Bass kernel for nn_MidLoss (segment-mean MSE loss).

Reference computation:
    seg_ids = repeat(arange(S), lengths)          # [N]
    means   = segment_sum(x, seg_ids) / lengths   # [S, D]
    loss    = mean((means[seg_ids] - x)**2)       # scalar

Algebraic identity (per segment s with rows x_i, L_s = len):
    sum_i ||x_i - mu_s||^2 = sum_i ||x_i||^2 - ||colsum_s||^2 / L_s
so the loss needs two sufficient statistics in ONE data pass:
    SSQ  = sum of x^2 over everything            (Gram diagonal)
    corr = sum_s ||sqrt(1/L_s) * colsum_s||^2    (weighted colsums)
    loss = (SSQ - corr) / (N * D)

Distribution: rows shard across 8 NeuronCores at segment boundaries.
Each core returns a partial (SSQ_c - corr_c); the scalar reduce is on host.

Device architecture (v2, all-PE fused DoubleRow):
  - Host prep (free): x cast fp32->fp8 e4m3 and laid out partition-major
    per tile with k=8 membership columns interleaved before every chunk:
    chunk image [128, k + 128] = [memb_w | x]. memb_w[p, j] =
    sqrt(1/L_j) if partition p's rows are in tile-segment j else 0, so
    the matmul directly produces 1/L-weighted colsums (corr is only
    ~0.2% of the loss; fp8 weight quantization error is negligible).
  - Per tile (G=32 chunks, 4096 rows): one SWDGE DMA streams the
    [128, G*(k+128)] fp8 image; the PE runs G/2 fused DoubleRow matmuls:
        lhsT = chunk pair [128, 2, 128], rhs = [memb|chunk] pair
        out  = psum [128, k+128] = [weighted colsums | Gram]
    One matmul per TWO chunks computes Gram_a+Gram_b (diag = SSQ) and
    the pair's weighted colsums at ~70-90 ns/pair (measured), leaving
    ScalarE idle and VectorE with only the per-tile psum drain.
  - Per tile drain (DVE, psum bank ping-pong): colsum cols copied to
    cs_all[:, kt:kt+k]; Gram cols added into a running fp32 gacc.
  - Endgame (DVE+PE): SSQ = reduce(gacc * I); corr = reduce(cs_all^2);
    partial = ones^T (ssq_vec - corr_vec); DMA out.

Measured: v1 (3-engine split) 88.2us; this version targets the DMA
stream floor (~17.8MB fp8 at ~410 GB/s sustained on one SWDGE queue).
"""

import os
import sys

for _p in ("/opt/trn_rl_repo", "/root/.axon_site/_ro/trn_rl_repo"):
    if os.path.isdir(_p) and _p not in sys.path:
        sys.path.insert(0, _p)

import numpy as np
import ml_dtypes

# Defensive: concourse's trace path imports antenv.axon_hooks, which some
# images lack. Provide the tiny get/set registry so a harness that enables
# BASS_TRACE does not crash; degrade silently if the hook source is absent.
try:
    import antenv.axon_hooks  # noqa: F401
except ImportError:
    try:
        import types as _types
        import antenv as _antenv
        _m = _types.ModuleType("antenv.axon_hooks")
        _h = [None]
        _m.set_axon_ntff_profile_hook = lambda v: _h.__setitem__(0, v)
        _m.get_axon_ntff_profile_hook = lambda: _h[0]
        _antenv.axon_hooks = _m
        sys.modules["antenv.axon_hooks"] = _m
        try:
            from trn_agent_boot.trn_boot import _ntff_profile_via_ctypes
            _m.set_axon_ntff_profile_hook(
                _ntff_profile_via_ctypes("/opt/axon/libaxon_pjrt.so"))
        except Exception:
            pass
    except Exception:
        pass

import concourse.bacc as bacc
import concourse.tile as tile
from concourse import mybir
from concourse.bass_utils import run_bass_kernel_spmd

N_CORES = 8
D = 128
G = int(os.environ.get("MIDLOSS_G", "32"))          # chunks per tile
DMA_BLOCK = int(os.environ.get("MIDLOSS_DMA_BLOCK", "1"))  # tiles per DMA
XB_BUFS = int(os.environ.get("MIDLOSS_BUFS", "6"))   # x tile pipeline depth
PSUM_BUFS = int(os.environ.get("MIDLOSS_PSUM_BUFS", "2"))
N_WARM = int(os.environ.get("MIDLOSS_WARM", "0"))    # PE warmup matmuls
N_SYNC_TILES = int(os.environ.get("MIDLOSS_SYNC_TILES", "0"))
XDMA_Q = os.environ.get("MIDLOSS_XDMA_Q", "gpsimd")
FILLER = int(os.environ.get("MIDLOSS_FILLER", "0"))
HALF_TILES = int(os.environ.get("MIDLOSS_HALF_TILES", "0"))
PRIME_KB = int(os.environ.get("MIDLOSS_PRIME_KB", "0"))  # priming DMA size


def _structure(lengths, n_cores=N_CORES, g=G):
    """Host-side plan. Returns (plan, fallback)."""
    lengths = np.asarray(lengths, dtype=np.int64)
    S = int(lengths.shape[0])
    offs = np.zeros(S + 1, dtype=np.int64)
    np.cumsum(lengths, out=offs[1:])
    N = int(offs[-1])

    if N % n_cores != 0:
        return None, True
    R = N // n_cores
    rows_tile = 128 * g
    if R % rows_tile != 0:
        return None, True
    T = R // rows_tile
    if np.any(lengths % g != 0):
        return None, True  # a partition's g rows would straddle segments

    # core and tile boundaries must all be segment boundaries
    bset = set(offs.tolist())
    for c in range(n_cores):
        for t in range(T + 1):
            if c * R + t * rows_tile not in bset:
                return None, True

    seg_of = {int(o): i for i, o in enumerate(offs)}
    cores = []
    k = None
    for c in range(n_cores):
        membs = []
        for t in range(T):
            base = c * R + t * rows_tile
            pstart = base + g * np.arange(128, dtype=np.int64)
            pseg = np.searchsorted(offs, pstart, side="right") - 1  # [128]
            s0 = seg_of[base]
            kt = int(pseg.max()) - s0 + 1
            if k is None:
                k = kt
            elif kt != k:
                return None, True
            w = np.sqrt(1.0 / lengths[pseg].astype(np.float64)).astype(np.float32)
            memb = np.zeros((128, kt), dtype=np.float32)
            memb[np.arange(128), pseg - s0] = w
            membs.append(memb)
        cores.append(dict(row_lo=c * R, row_hi=(c + 1) * R, membs=membs))
    if k > 32 or (g % 2) != 0:
        return None, True
    C = k + D
    if ((g // 2) * C) % 16 != 0:
        return None, True  # DoubleRow weight-load pair-stride restriction
    if HALF_TILES > 0 and (((g // 4) * C) % 16 != 0 or (g % 4) != 0):
        return None, True  # intra-half pairing stride for the split first tile
    if T % DMA_BLOCK != 0:
        return None, True
    plan = dict(R=R, T=T, g=g, k=k, C=C, N=N, cores=cores)
    return plan, False


def _sig(plan):
    return (plan["R"], plan["T"], plan["g"], plan["k"])


def _build_nc(plan):
    f32 = mybir.dt.float32
    xdt = mybir.dt.float8e4
    T, g, k, C = plan["T"], plan["g"], plan["k"], plan["C"]
    DR = mybir.MatmulPerfMode.DoubleRow

    nc = bacc.Bacc()
    x = nc.dram_tensor("x", [128, T * g * C], xdt, kind="ExternalInput")
    ident = nc.dram_tensor("ident", [128, 128], xdt, kind="ExternalInput")
    y = nc.dram_tensor("y", [1, 1], f32, kind="ExternalOutput")

    with tile.TileContext(nc) as tc:
        with (
            tc.tile_pool(name="xb", bufs=XB_BUFS) as xb_pool,
            tc.tile_pool(name="small", bufs=1) as small,
            tc.tile_pool(name="psum", bufs=PSUM_BUFS, space="PSUM") as psum_pool,
            tc.tile_pool(name="psmisc", bufs=1, space="PSUM") as psum_misc,
        ):
            ident_sb = small.tile([128, 128], xdt)
            nc.sync.dma_start(out=ident_sb[:], in_=ident[:])
            ones_sb = small.tile([128, 1], f32)
            nc.vector.memset(ones_sb[:], 1.0)
            gacc = small.tile([128, 128], f32)
            nc.vector.memset(gacc[:], 0.0)
            cs_all = small.tile([128, T * k], f32)

            if PRIME_KB > 0:
                prime = small.tile([128, PRIME_KB * 8], xdt)
                nc.gpsimd.dma_start(out=prime[:], in_=x[:, 0:PRIME_KB * 8])

            # PE warmup / activity filler scratch (HAM clock gate: the PE
            # drops to 1.2 GHz after an idle 3.4us window; filler matmuls
            # between tiles keep the activity monitor busy).
            warm = small.tile([128, 256], xdt)
            nc.vector.memset(warm[:], 0.0)
            warm_v = warm[:].rearrange("p (two f) -> p two f", two=2)
            psum_w = psum_misc.tile([128, 128], f32)
            for i in range(N_WARM):
                nc.tensor.matmul(psum_w[:], lhsT=warm_v, rhs=warm_v,
                                 start=(i == 0), stop=(i == N_WARM - 1),
                                 perf_mode=DR)
            # warm the ACT Square table during the preamble so the endgame
            # ACTIVATE does not pay the ~1.3us table load
            act_scratch = small.tile([128, T * k], mybir.dt.bfloat16)
            act_acc = small.tile([128, 1], f32)
            nc.scalar.activation(out=act_scratch[:, 0:1], in_=ones_sb[:],
                                 func=mybir.ActivationFunctionType.Square,
                                 accum_out=act_acc[:])

            BT = DMA_BLOCK
            tile_w = g * C
            for b in range(T // BT):
                xb = xb_pool.tile([128, BT * tile_w], xdt)
                if XDMA_Q == "scalar":
                    dma_eng = nc.scalar
                elif XDMA_Q == "sync":
                    dma_eng = nc.sync
                else:
                    dma_eng = nc.gpsimd
                base_c = b * BT * tile_w
                if b < HALF_TILES and BT == 1:
                    # split the first tile's DMA so the PE can start on the
                    # first half ~0.7us earlier (pairs are intra-half there)
                    hw_ = tile_w // 2
                    dma_eng.dma_start(out=xb[:, 0:hw_],
                                      in_=x[:, base_c:base_c + hw_])
                    dma_eng.dma_start(out=xb[:, hw_:tile_w],
                                      in_=x[:, base_c + hw_:base_c + tile_w])
                else:
                    dma_eng.dma_start(out=xb[:],
                                      in_=x[:, base_c:(b + 1) * BT * tile_w])
                for tt in range(BT):
                    t = b * BT + tt
                    # pair chunk i with chunk i+g/2: the DoubleRow weight-load
                    # requires the pair stride to be a multiple of 16 elements;
                    # (g/2)*C satisfies that while adjacent chunks (stride C)
                    # do not. Both pair members share the tile's memb columns.
                    ps = psum_pool.tile([128, C], f32)
                    if b < HALF_TILES and BT == 1:
                        # intra-half pairing (i, i+g/4): pair stride
                        # (g/4)*C must also be a multiple of 16
                        for h_ in range(2):
                            hv = xb[:, h_ * tile_w // 2:(h_ + 1) * tile_w // 2]
                            hv = hv.rearrange("p (h g c) -> p h g c", h=2, c=C)
                            for i in range(g // 4):
                                nc.tensor.matmul(
                                    ps[:],
                                    lhsT=hv[:, :, i, k:C],
                                    rhs=hv[:, :, i, 0:C],
                                    start=(h_ == 0 and i == 0),
                                    stop=(h_ == 1 and i == g // 4 - 1),
                                    perf_mode=DR,
                                )
                    else:
                        v = xb[:, tt * tile_w:(tt + 1) * tile_w].rearrange(
                            "p (h g c) -> p h g c", h=2, c=C)
                        for i in range(g // 2):
                            nc.tensor.matmul(
                                ps[:],
                                lhsT=v[:, :, i, k:C],
                                rhs=v[:, :, i, 0:C],
                                start=(i == 0), stop=(i == g // 2 - 1),
                                perf_mode=DR,
                            )
                    for _f in range(FILLER if t < T - 1 else 0):
                        nc.tensor.matmul(psum_w[:], lhsT=warm_v, rhs=warm_v,
                                         start=True, stop=True, perf_mode=DR)
                    nc.vector.tensor_copy(out=cs_all[:, t * k:(t + 1) * k],
                                          in_=ps[:, 0:k])
                    nc.vector.tensor_add(gacc[:], gacc[:], ps[:, k:C])

            # ---- endgame ----
            gmask = small.tile([128, 128], f32)
            nc.vector.tensor_mul(gmask[:], gacc[:], ident_sb[:])
            ssqv = small.tile([128, 1], f32)
            nc.vector.tensor_reduce(out=ssqv[:], in_=gmask[:],
                                    axis=mybir.AxisListType.X,
                                    op=mybir.AluOpType.add)
            corrv = small.tile([128, 1], f32)
            nc.scalar.activation(out=act_scratch[:], in_=cs_all[:],
                                 func=mybir.ActivationFunctionType.Square,
                                 accum_out=corrv[:])
            diff = small.tile([128, 1], f32)
            nc.vector.tensor_sub(diff[:], ssqv[:], corrv[:])
            psum_s = psum_misc.tile([1, 1], f32)
            nc.tensor.matmul(psum_s[:], lhsT=ones_sb[:], rhs=diff[:],
                             start=True, stop=True)
            out_sb = small.tile([1, 1], f32)
            nc.vector.tensor_copy(out=out_sb[:], in_=psum_s[:])
            nc.sync.dma_start(out=y[:], in_=out_sb[:])

    nc.compile()
    return nc


_CACHE = {}


def _get_nc(plan):
    key = (_sig(plan), XB_BUFS, PSUM_BUFS, N_WARM, N_SYNC_TILES, XDMA_Q, FILLER,
           DMA_BLOCK, HALF_TILES, PRIME_KB)
    nc = _CACHE.get(key)
    if nc is None:
        nc = _build_nc(plan)
        _CACHE[key] = nc
    return nc


def _host_images(plan, x_np):
    """Per-core interleaved fp8 images [128, T*g*C]: [memb_w | x] per chunk."""
    T, g, k, C, R = plan["T"], plan["g"], plan["k"], plan["C"], plan["R"]
    fp8 = ml_dtypes.float8_e4m3
    images = []
    for c in range(N_CORES):
        info = plan["cores"][c]
        xc = x_np[info["row_lo"]:info["row_hi"]]              # [R, D] fp32
        x4 = np.ascontiguousarray(xc).reshape(T, 128, g, D)
        xi = np.empty((128, T, g, C), dtype=fp8)
        memb = np.stack(info["membs"], axis=0)                # [T, 128, k]
        xi[:, :, :, :k] = memb.astype(fp8).transpose(1, 0, 2)[:, :, None, :]
        xi[:, :, :, k:] = x4.astype(fp8).transpose(1, 0, 2, 3)
        images.append(np.ascontiguousarray(xi.reshape(128, T * g * C)))
    return images


def _run_spmd(plan, x_np, trace=False):
    nc = _get_nc(plan)
    ident = np.eye(128, dtype=np.float32).astype(ml_dtypes.float8_e4m3)
    in_maps = [{"x": img, "ident": ident} for img in _host_images(plan, x_np)]
    last_err = None
    for attempt in range(3):
        try:
            res = run_bass_kernel_spmd(nc, in_maps,
                                       core_ids=list(range(N_CORES)),
                                       trace=trace)
            break
        except Exception as e:  # rare transient device flakes
            last_err = e
    else:
        raise last_err
    partials = [float(res.results[c]["y"][0, 0]) for c in range(N_CORES)]
    return partials, res


def _numpy_fallback(x_np, lengths):
    """Pure-host fallback for input structures the SPMD path can't express."""
    lengths = np.asarray(lengths, dtype=np.int64)
    offs = np.concatenate([[0], np.cumsum(lengths)])
    x = x_np.astype(np.float64)
    ssq = float((x * x).sum())
    corr = 0.0
    for s in range(len(lengths)):
        cs = x[offs[s]:offs[s + 1]].sum(axis=0)
        corr += float((cs * cs).sum()) / float(lengths[s])
    return np.float32((ssq - corr) / x.size)


def kernel(inputs, lengths):
    x_np = np.asarray(inputs, dtype=np.float32)
    lengths_np = np.asarray(lengths)
    plan, fallback = _structure(lengths_np)
    if fallback:
        return _numpy_fallback(x_np, lengths_np)
    partials, _ = _run_spmd(plan, x_np)
    total = float(np.sum(np.asarray(partials, dtype=np.float64)))
    loss = total / (plan["N"] * D)
    return np.asarray(loss, dtype=np.float32)


# revision 17
# speedup vs baseline: 1.0394x; 1.0394x over previous
"""Trainium2 Bass kernel for nn_MidLoss (segment-mean MSE loss).

Reference computation:
    seg_ids = repeat(arange(S), lengths)          # [N]
    means   = segment_sum(x, seg_ids) / lengths   # [S, D]
    loss    = mean((means[seg_ids] - x)**2)       # scalar

Algebraic identity (per segment s with rows x_i, L_s = len):
    sum_i ||x_i - mu_s||^2 = sum_i ||x_i||^2 - ||colsum_s||^2 / L_s
so the loss needs two sufficient statistics in ONE data pass:
    SSQ  = sum of x^2 over everything            (Gram diagonal)
    corr = sum_s ||sqrt(1/L_s) * colsum_s||^2    (weighted colsums)
    loss = (SSQ - corr) / (N * D)

Distribution: rows shard across 8 NeuronCores at segment boundaries.
Each core returns a partial (SSQ_c - corr_c); the scalar reduce is on host.

Device architecture (v2, all-PE fused DoubleRow):
  - Host prep (free): x cast fp32->fp8 e4m3 and laid out partition-major
    per tile with k=8 membership columns interleaved before every chunk:
    chunk image [128, k + 128] = [memb_w | x]. memb_w[p, j] =
    sqrt(1/L_j) if partition p's rows are in tile-segment j else 0, so
    the matmul directly produces 1/L-weighted colsums (corr is only
    ~0.2% of the loss; fp8 weight quantization error is negligible).
  - Per tile (G=32 chunks, 4096 rows): one SWDGE DMA streams the
    [128, G*(k+128)] fp8 image; the PE runs G/2 fused DoubleRow matmuls:
        lhsT = chunk pair [128, 2, 128], rhs = [memb|chunk] pair
        out  = psum [128, k+128] = [weighted colsums | Gram]
    One matmul per TWO chunks computes Gram_a+Gram_b (diag = SSQ) and
    the pair's weighted colsums at ~70-90 ns/pair (measured), leaving
    ScalarE idle and VectorE with only the per-tile psum drain.
  - Per tile drain (DVE, psum bank ping-pong): colsum cols copied to
    cs_all[:, kt:kt+k]; Gram cols added into a running fp32 gacc.
  - Endgame (DVE+PE): SSQ = reduce(gacc * I); corr = reduce(cs_all^2);
    partial = ones^T (ssq_vec - corr_vec); DMA out.

Measured: v1 (3-engine split) 88.2us; this version targets the DMA
stream floor (~17.8MB fp8 at ~410 GB/s sustained on one SWDGE queue).
"""

import os
import sys

for _p in ("/opt/trn_rl_repo", "/root/.axon_site/_ro/trn_rl_repo"):
    if os.path.isdir(_p) and _p not in sys.path:
        sys.path.insert(0, _p)

import numpy as np
import ml_dtypes

# Defensive: concourse's trace path imports antenv.axon_hooks, which some
# images lack. Provide the tiny get/set registry so a harness that enables
# BASS_TRACE does not crash; degrade silently if the hook source is absent.
try:
    import antenv.axon_hooks  # noqa: F401
except ImportError:
    try:
        import types as _types
        import antenv as _antenv
        _m = _types.ModuleType("antenv.axon_hooks")
        _h = [None]
        _m.set_axon_ntff_profile_hook = lambda v: _h.__setitem__(0, v)
        _m.get_axon_ntff_profile_hook = lambda: _h[0]
        _antenv.axon_hooks = _m
        sys.modules["antenv.axon_hooks"] = _m
        try:
            from trn_agent_boot.trn_boot import _ntff_profile_via_ctypes
            _m.set_axon_ntff_profile_hook(
                _ntff_profile_via_ctypes("/opt/axon/libaxon_pjrt.so"))
        except Exception:
            pass
    except Exception:
        pass

import concourse.bacc as bacc
import concourse.tile as tile
from concourse import mybir
from concourse.bass_utils import run_bass_kernel_spmd

N_CORES = 8
D = 128
G = int(os.environ.get("MIDLOSS_G", "32"))          # chunks per tile
DMA_BLOCK = int(os.environ.get("MIDLOSS_DMA_BLOCK", "1"))  # tiles per DMA
XB_BUFS = int(os.environ.get("MIDLOSS_BUFS", "8"))   # x tile pipeline depth
PSUM_BUFS = int(os.environ.get("MIDLOSS_PSUM_BUFS", "3"))
N_WARM = int(os.environ.get("MIDLOSS_WARM", "0"))    # PE warmup matmuls
N_SYNC_TILES = int(os.environ.get("MIDLOSS_SYNC_TILES", "0"))
XDMA_Q = os.environ.get("MIDLOSS_XDMA_Q", "gpsimd")
FILLER = int(os.environ.get("MIDLOSS_FILLER", "0"))
HALF_TILES = int(os.environ.get("MIDLOSS_HALF_TILES", "0"))
PRIME_KB = int(os.environ.get("MIDLOSS_PRIME_KB", "0"))  # priming DMA size


def _structure(lengths, n_cores=N_CORES, g=G):
    """Host-side plan. Returns (plan, fallback)."""
    lengths = np.asarray(lengths, dtype=np.int64)
    S = int(lengths.shape[0])
    offs = np.zeros(S + 1, dtype=np.int64)
    np.cumsum(lengths, out=offs[1:])
    N = int(offs[-1])

    if N % n_cores != 0:
        return None, True
    R = N // n_cores
    rows_tile = 128 * g
    if R % rows_tile != 0:
        return None, True
    T = R // rows_tile
    if np.any(lengths % g != 0):
        return None, True  # a partition's g rows would straddle segments

    # core and tile boundaries must all be segment boundaries
    bset = set(offs.tolist())
    for c in range(n_cores):
        for t in range(T + 1):
            if c * R + t * rows_tile not in bset:
                return None, True

    seg_of = {int(o): i for i, o in enumerate(offs)}
    cores = []
    k = None
    for c in range(n_cores):
        membs = []
        for t in range(T):
            base = c * R + t * rows_tile
            pstart = base + g * np.arange(128, dtype=np.int64)
            pseg = np.searchsorted(offs, pstart, side="right") - 1  # [128]
            s0 = seg_of[base]
            kt = int(pseg.max()) - s0 + 1
            if k is None:
                k = kt
            elif kt != k:
                return None, True
            w = np.sqrt(1.0 / lengths[pseg].astype(np.float64)).astype(np.float32)
            memb = np.zeros((128, kt), dtype=np.float32)
            memb[np.arange(128), pseg - s0] = w
            membs.append(memb)
        cores.append(dict(row_lo=c * R, row_hi=(c + 1) * R, membs=membs))
    if k > 32 or (g % 2) != 0:
        return None, True
    C = k + D
    if ((g // 2) * C) % 16 != 0:
        return None, True  # DoubleRow weight-load pair-stride restriction
    if HALF_TILES > 0 and (((g // 4) * C) % 16 != 0 or (g % 4) != 0):
        return None, True  # intra-half pairing stride for the split first tile
    if T % DMA_BLOCK != 0:
        return None, True
    plan = dict(R=R, T=T, g=g, k=k, C=C, N=N, cores=cores)
    return plan, False


def _sig(plan):
    return (plan["R"], plan["T"], plan["g"], plan["k"])


def _build_nc(plan):
    f32 = mybir.dt.float32
    xdt = mybir.dt.float8e4
    T, g, k, C = plan["T"], plan["g"], plan["k"], plan["C"]
    DR = mybir.MatmulPerfMode.DoubleRow

    nc = bacc.Bacc()
    x = nc.dram_tensor("x", [128, T * g * C], xdt, kind="ExternalInput")
    ident = nc.dram_tensor("ident", [128, 128], xdt, kind="ExternalInput")
    y = nc.dram_tensor("y", [1, 1], f32, kind="ExternalOutput")

    with tile.TileContext(nc) as tc:
        with (
            tc.tile_pool(name="xb", bufs=XB_BUFS) as xb_pool,
            tc.tile_pool(name="small", bufs=1) as small,
            tc.tile_pool(name="psum", bufs=PSUM_BUFS, space="PSUM") as psum_pool,
            tc.tile_pool(name="psmisc", bufs=1, space="PSUM") as psum_misc,
        ):
            ident_sb = small.tile([128, 128], xdt)
            nc.sync.dma_start(out=ident_sb[:], in_=ident[:])
            ones_sb = small.tile([128, 1], f32)
            nc.vector.memset(ones_sb[:], 1.0)
            gacc = small.tile([128, 128], f32)
            nc.vector.memset(gacc[:], 0.0)
            cs_all = small.tile([128, T * k], f32)

            if PRIME_KB > 0:
                prime = small.tile([128, PRIME_KB * 8], xdt)
                nc.gpsimd.dma_start(out=prime[:], in_=x[:, 0:PRIME_KB * 8])

            # PE warmup / activity filler scratch (HAM clock gate: the PE
            # drops to 1.2 GHz after an idle 3.4us window; filler matmuls
            # between tiles keep the activity monitor busy).
            warm = small.tile([128, 256], xdt)
            nc.vector.memset(warm[:], 0.0)
            warm_v = warm[:].rearrange("p (two f) -> p two f", two=2)
            psum_w = psum_misc.tile([128, 128], f32)
            for i in range(N_WARM):
                nc.tensor.matmul(psum_w[:], lhsT=warm_v, rhs=warm_v,
                                 start=(i == 0), stop=(i == N_WARM - 1),
                                 perf_mode=DR)
            # warm the ACT Square table during the preamble so the endgame
            # ACTIVATE does not pay the ~1.3us table load
            act_scratch = small.tile([128, T * k], mybir.dt.bfloat16)
            act_acc = small.tile([128, 1], f32)
            nc.scalar.activation(out=act_scratch[:, 0:1], in_=ones_sb[:],
                                 func=mybir.ActivationFunctionType.Square,
                                 accum_out=act_acc[:])

            BT = DMA_BLOCK
            tile_w = g * C
            for b in range(T // BT):
                xb = xb_pool.tile([128, BT * tile_w], xdt)
                if XDMA_Q == "scalar":
                    dma_eng = nc.scalar
                elif XDMA_Q == "sync":
                    dma_eng = nc.sync
                else:
                    dma_eng = nc.gpsimd
                base_c = b * BT * tile_w
                if b < HALF_TILES and BT == 1:
                    # split the first tile's DMA so the PE can start on the
                    # first half ~0.7us earlier (pairs are intra-half there)
                    hw_ = tile_w // 2
                    dma_eng.dma_start(out=xb[:, 0:hw_],
                                      in_=x[:, base_c:base_c + hw_])
                    dma_eng.dma_start(out=xb[:, hw_:tile_w],
                                      in_=x[:, base_c + hw_:base_c + tile_w])
                else:
                    dma_eng.dma_start(out=xb[:],
                                      in_=x[:, base_c:(b + 1) * BT * tile_w])
                for tt in range(BT):
                    t = b * BT + tt
                    # pair chunk i with chunk i+g/2: the DoubleRow weight-load
                    # requires the pair stride to be a multiple of 16 elements;
                    # (g/2)*C satisfies that while adjacent chunks (stride C)
                    # do not. Both pair members share the tile's memb columns.
                    ps = psum_pool.tile([128, C], f32)
                    if b < HALF_TILES and BT == 1:
                        # intra-half pairing (i, i+g/4): pair stride
                        # (g/4)*C must also be a multiple of 16
                        for h_ in range(2):
                            hv = xb[:, h_ * tile_w // 2:(h_ + 1) * tile_w // 2]
                            hv = hv.rearrange("p (h g c) -> p h g c", h=2, c=C)
                            for i in range(g // 4):
                                nc.tensor.matmul(
                                    ps[:],
                                    lhsT=hv[:, :, i, k:C],
                                    rhs=hv[:, :, i, 0:C],
                                    start=(h_ == 0 and i == 0),
                                    stop=(h_ == 1 and i == g // 4 - 1),
                                    perf_mode=DR,
                                )
                    else:
                        v = xb[:, tt * tile_w:(tt + 1) * tile_w].rearrange(
                            "p (h g c) -> p h g c", h=2, c=C)
                        for i in range(g // 2):
                            nc.tensor.matmul(
                                ps[:],
                                lhsT=v[:, :, i, k:C],
                                rhs=v[:, :, i, 0:C],
                                start=(i == 0), stop=(i == g // 2 - 1),
                                perf_mode=DR,
                            )
                    for _f in range(FILLER if t < T - 1 else 0):
                        nc.tensor.matmul(psum_w[:], lhsT=warm_v, rhs=warm_v,
                                         start=True, stop=True, perf_mode=DR)
                    nc.vector.tensor_copy(out=cs_all[:, t * k:(t + 1) * k],
                                          in_=ps[:, 0:k])
                    nc.vector.tensor_add(gacc[:], gacc[:], ps[:, k:C])

            # ---- endgame ----
            gmask = small.tile([128, 128], f32)
            nc.vector.tensor_mul(gmask[:], gacc[:], ident_sb[:])
            ssqv = small.tile([128, 1], f32)
            nc.vector.tensor_reduce(out=ssqv[:], in_=gmask[:],
                                    axis=mybir.AxisListType.X,
                                    op=mybir.AluOpType.add)
            corrv = small.tile([128, 1], f32)
            nc.scalar.activation(out=act_scratch[:], in_=cs_all[:],
                                 func=mybir.ActivationFunctionType.Square,
                                 accum_out=corrv[:])
            diff = small.tile([128, 1], f32)
            nc.vector.tensor_sub(diff[:], ssqv[:], corrv[:])
            psum_s = psum_misc.tile([1, 1], f32)
            nc.tensor.matmul(psum_s[:], lhsT=ones_sb[:], rhs=diff[:],
                             start=True, stop=True)
            out_sb = small.tile([1, 1], f32)
            nc.vector.tensor_copy(out=out_sb[:], in_=psum_s[:])
            nc.sync.dma_start(out=y[:], in_=out_sb[:])

    nc.compile()
    return nc


_CACHE = {}


def _get_nc(plan):
    key = (_sig(plan), XB_BUFS, PSUM_BUFS, N_WARM, N_SYNC_TILES, XDMA_Q, FILLER,
           DMA_BLOCK, HALF_TILES, PRIME_KB)
    nc = _CACHE.get(key)
    if nc is None:
        nc = _build_nc(plan)
        _CACHE[key] = nc
    return nc


def _host_images(plan, x_np):
    """Per-core interleaved fp8 images [128, T*g*C]: [memb_w | x] per chunk."""
    T, g, k, C, R = plan["T"], plan["g"], plan["k"], plan["C"], plan["R"]
    fp8 = ml_dtypes.float8_e4m3
    images = []
    for c in range(N_CORES):
        info = plan["cores"][c]
        xc = x_np[info["row_lo"]:info["row_hi"]]              # [R, D] fp32
        x4 = np.ascontiguousarray(xc).reshape(T, 128, g, D)
        xi = np.empty((128, T, g, C), dtype=fp8)
        memb = np.stack(info["membs"], axis=0)                # [T, 128, k]
        xi[:, :, :, :k] = memb.astype(fp8).transpose(1, 0, 2)[:, :, None, :]
        xi[:, :, :, k:] = x4.astype(fp8).transpose(1, 0, 2, 3)
        images.append(np.ascontiguousarray(xi.reshape(128, T * g * C)))
    return images


def _run_spmd(plan, x_np, trace=False):
    nc = _get_nc(plan)
    ident = np.eye(128, dtype=np.float32).astype(ml_dtypes.float8_e4m3)
    in_maps = [{"x": img, "ident": ident} for img in _host_images(plan, x_np)]
    last_err = None
    for attempt in range(3):
        try:
            res = run_bass_kernel_spmd(nc, in_maps,
                                       core_ids=list(range(N_CORES)),
                                       trace=trace)
            break
        except Exception as e:  # rare transient device flakes
            last_err = e
    else:
        raise last_err
    partials = [float(res.results[c]["y"][0, 0]) for c in range(N_CORES)]
    return partials, res


def _numpy_fallback(x_np, lengths):
    """Pure-host fallback for input structures the SPMD path can't express."""
    lengths = np.asarray(lengths, dtype=np.int64)
    offs = np.concatenate([[0], np.cumsum(lengths)])
    x = x_np.astype(np.float64)
    ssq = float((x * x).sum())
    corr = 0.0
    for s in range(len(lengths)):
        cs = x[offs[s]:offs[s + 1]].sum(axis=0)
        corr += float((cs * cs).sum()) / float(lengths[s])
    return np.float32((ssq - corr) / x.size)


def kernel(inputs, lengths):
    x_np = np.asarray(inputs, dtype=np.float32)
    lengths_np = np.asarray(lengths)
    plan, fallback = _structure(lengths_np)
    if fallback:
        return _numpy_fallback(x_np, lengths_np)
    partials, _ = _run_spmd(plan, x_np)
    total = float(np.sum(np.asarray(partials, dtype=np.float64)))
    loss = total / (plan["N"] * D)
    return np.asarray(loss, dtype=np.float32)
